# revision 12
# baseline (speedup 1.0000x reference)
"""Trainium2 Bass kernel for nn_DistanceHead (gnn_message_passing).

Pipeline per edge e:  vals[e] = LN(gelu(dist[e] @ W1 + b1)) @ (gamma*W2) + ...
followed by to_undirected coalesce (sort/unique over linearized edge ids).

Device strategy (8 NeuronCores, data-parallel over E):
  - Orientation "B": features (D=128) on partitions, edges on the free dim.
  - dist is pre-transposed on host into A_core[128, Epq]: row 32*q+k holds
    feature k of edge-quarter q. One DMA per round loads [128, 512] (2048
    edges = 4 quarters x 512).
  - mm1: out.T = W1.T @ dist.T per quarter via K=32 row-tiled matmuls
    (tile_position=(32q, 0)) with W1 replicated into all 4 row groups.
  - ACT applies erf-Gelu (+b1 per-partition bias) PSUM->SBUF.
  - DVE squares g (for the variance second moment).
  - LayerNorm + final dot are folded into three reductions over features:
    sum(g), dot(g, gamma*W2), sum(g^2), computed on the PE as matmuls with
    stationary [ones | gamma*W2] weights (M=2, col-tiled 4x).
  - Per-round stat rows are copied PSUM->SBUF (lane-preserving) and
    re-spread to a [128, F] per-edge layout with SBUF->SBUF DMAs.
  - Final assembly: vals = rstd*(dot - mu*S) + (beta@W2 + b2), computed once
    on [128, F] tiles.
Host does: input re-layout/sharding, gamma/beta folding, output un-permute,
and the global coalesce (sort/unique/segment-mean over 4M int64 ids).
"""

import numpy as np

import concourse.bass as bass
import concourse.mybir as mybir
from concourse.tile import TileContext
from concourse.vector_clock import ScopedClock
from concourse.bass_utils import run_bass_kernel_spmd

N_NODES = 100000
EPS = 1e-5
N_CORES = 8
H = 32          # input feature dim
D = 128         # hidden dim
NQ = 512        # edges per quarter per round
RQ = 4          # quarters (128 partitions / 32 features)
ROUND_E = NQ * RQ  # 2048 edges per round


# ---------------------------------------------------------------------------
# TileContext exit-drain fix: this toolchain's walrus rejects >1 sync wait on
# a Drain instruction ("Too many sync wait commands"). Split the exit drain's
# waits across one drain per semaphore.
def _drain_and_barrier_split(self, tick_clock, wait_clock):
    drain_inst = self.nc.sync.drain()
    wait_clock.add_sem_waits(
        drain_inst.ins, ScopedClock({None: tick_clock.global_clock})
    )
    si = drain_inst.ins.sync_info
    waits = list(si.on_wait) if si is not None else []
    if len(waits) > 1:
        drain_inst.ins.sync_info = mybir.SyncInfo(
            on_update=list(si.on_update), on_wait=[waits[0]]
        )
        for w in waits[1:]:
            extra = self.nc.sync.drain()
            extra.ins.sync_info = mybir.SyncInfo(on_update=[], on_wait=[w])
    self.nc.all_engine_barrier()
    popped = self.nc._tile_sem_poison_stack.pop()
    assert popped is self._sem_poison
    self.nc.clear_and_free_semaphores(list(self.sems.allocated().values()))
    self.nc.all_engine_barrier()


class _TileContextSplitDrain(TileContext):
    _drain_and_barrier = _drain_and_barrier_split


def _split_multi_waits(mod: dict) -> dict:
    """This walrus build rejects instructions carrying more than one sync
    wait ("Too many sync wait commands"). Hoist extra waits onto wait-only
    EventSemaphore instructions inserted just before, on the same engine
    queue (queues dispatch in order, so semantics are preserved)."""
    n = 0
    for f in mod["functions"]:
        for blk in f["blocks"]:
            new_insts = []
            for inst in blk["instructions"]:
                si = inst.get("sync_info")
                waits = (si or {}).get("on_wait") or []
                if len(waits) > 1:
                    for j, w in enumerate(waits[:-1]):
                        n += 1
                        new_insts.append({
                            "name": f"{inst['name']}_xw{j}",
                            "opcode": "EventSemaphore",
                            "engine": inst["engine"],
                            "ins": [],
                            "outs": [],
                            "debug": inst.get("debug"),
                            "sync_info": {"on_update": [], "on_wait": [w]},
                        })
                    si["on_wait"] = [waits[-1]]
                new_insts.append(inst)
            blk["instructions"] = new_insts
    return mod


def _patch_serialization(nc: bass.Bass) -> None:
    import orjson

    orig = nc.to_json_bytes

    def to_json_bytes_patched():
        return orjson.dumps(_split_multi_waits(orjson.loads(orig())))

    nc.to_json_bytes = to_json_bytes_patched


# ---------------------------------------------------------------------------
def build_mlp_kernel(n_rounds: int) -> bass.Bass:
    """Device kernel for one core: Epq = n_rounds*NQ edges per quarter."""
    f32 = mybir.dt.float32
    AF = mybir.ActivationFunctionType
    OP = mybir.AluOpType
    Epq = n_rounds * NQ
    F = n_rounds * ROUND_E // 128  # free size of per-edge stat buffers

    nc = bass.Bass()
    a_in = nc.dram_tensor("a", [128, Epq], f32, kind="ExternalInput")
    w1rep_in = nc.dram_tensor("w1rep", [128, D], f32, kind="ExternalInput")
    cw_in = nc.dram_tensor("cw", [128, 2], f32, kind="ExternalInput")
    b1_in = nc.dram_tensor("b1t", [128, 1], f32, kind="ExternalInput")
    par_in = nc.dram_tensor("params", [128, 2], f32, kind="ExternalInput")
    vals_out = nc.dram_tensor("vals", [128, F], f32, kind="ExternalOutput")

    with _TileContextSplitDrain(nc) as tc:
        with (
            tc.tile_pool(name="consts", bufs=1) as cpool,
            tc.tile_pool(name="din", bufs=4) as dpool,
            tc.tile_pool(name="gbuf", bufs=2) as gpool,
            tc.tile_pool(name="g2buf", bufs=2) as g2pool,
            tc.tile_pool(name="stage", bufs=2) as spool,
            tc.tile_pool(name="stats", bufs=1) as stpool,
            tc.tile_pool(name="asm", bufs=1) as apool,
            tc.tile_pool(name="hp", bufs=3, space="PSUM") as hpool,
            tc.tile_pool(name="pstat", bufs=1, space="PSUM") as pspool,
        ):
            # Constants
            w1rep = cpool.tile([128, D], f32)
            nc.sync.dma_start(w1rep[:], w1rep_in[:])
            cw = cpool.tile([128, 2], f32)
            nc.sync.dma_start(cw[:], cw_in[:])
            b1t = cpool.tile([128, 1], f32)
            nc.sync.dma_start(b1t[:], b1_in[:])
            params = cpool.tile([128, 2], f32)
            nc.sync.dma_start(params[:], par_in[:])

            # Per-edge stat accumulation buffers [128, F]
            sumb = stpool.tile([128, F], f32, tag="sumb")
            dotb = stpool.tile([128, F], f32, tag="dotb")
            sqb = stpool.tile([128, F], f32, tag="sqb")

            for r in range(n_rounds):
                # ---- load dist tile [128, 512]: 4 quarters x 512 edges
                dt = dpool.tile([128, NQ], f32, tag="din")
                nc.sync.dma_start(dt[:], a_in[:, r * NQ:(r + 1) * NQ])

                # ---- mm1: h.T per quarter (row-tiled K=32 matmuls)
                hpA = hpool.tile([128, 2 * NQ], f32, tag="hp")  # quarters 0,1
                hpB = hpool.tile([128, 2 * NQ], f32, tag="hp")  # quarters 2,3
                for q in range(RQ):
                    hp = hpA if q < 2 else hpB
                    col = (q % 2) * NQ
                    nc.tensor.matmul(
                        hp[:, col:col + NQ],
                        lhsT=w1rep[32 * q:32 * (q + 1), :],
                        rhs=dt[32 * q:32 * (q + 1), :],
                        tile_position=(32 * q, 0),
                    )

                # ---- gelu (+b1) PSUM -> SBUF, f32
                gt = gpool.tile([128, ROUND_E], f32, tag="gt")
                nc.scalar.activation(
                    gt[:, 0:2 * NQ], hpA[:], AF.Gelu, bias=b1t[:, 0:1]
                )
                nc.scalar.activation(
                    gt[:, 2 * NQ:4 * NQ], hpB[:], AF.Gelu, bias=b1t[:, 0:1]
                )

                # ---- square for variance
                g2t = g2pool.tile([128, ROUND_E], f32, tag="g2t")
                nc.vector.tensor_tensor(
                    out=g2t[:], in0=gt[:], in1=gt[:], op=OP.mult
                )

                # ---- stat matmuls: [ones | gamma*W2].T @ g -> [2, 512] per
                # quarter, col-tiled into one PSUM bank.
                gstat = pspool.tile([128, NQ], f32, tag="gstat")
                g2stat = pspool.tile([128, NQ], f32, tag="g2stat")
                for c in range(RQ):
                    nc.tensor.matmul(
                        gstat[32 * c:32 * c + 2, :],
                        lhsT=cw[:, 0:2],
                        rhs=gt[:, NQ * c:NQ * (c + 1)],
                        tile_position=(0, 32 * c),
                    )
                    nc.tensor.matmul(
                        g2stat[32 * c:32 * c + 1, :],
                        lhsT=cw[:, 0:1],
                        rhs=g2t[:, NQ * c:NQ * (c + 1)],
                        tile_position=(0, 32 * c),
                    )

                # ---- copy stat rows PSUM -> SBUF (lane-preserving)
                sg = spool.tile([128, NQ], f32, tag="sg")
                s2g = spool.tile([128, NQ], f32, tag="s2g")
                # sum rows {0,32,64,96}, dot rows {1,33,65,97}. Engines
                # need partition step 1, so copy the contiguous row range
                # (cost is free-dim driven; extra lanes are free).
                nc.scalar.copy(sg[0:98, :], gstat[0:98, :])
                nc.vector.tensor_copy(s2g[0:97, :], g2stat[0:97, :])

                # ---- re-spread to per-edge layout [128, 16] via SBUF DMA
                fo = r * (ROUND_E // 128)
                fw = ROUND_E // 128
                nc.gpsimd.dma_start(sumb[:, fo:fo + fw], sg[0:128:32, :])
                nc.gpsimd.dma_start(dotb[:, fo:fo + fw], sg[1:128:32, :])
                nc.gpsimd.dma_start(sqb[:, fo:fo + fw], s2g[0:128:32, :])

            # ---- final assembly on [128, F]
            msq = apool.tile([128, F], f32, tag="msq")
            nc.vector.tensor_tensor(out=msq[:], in0=sumb[:], in1=sumb[:], op=OP.mult)
            v1 = apool.tile([128, F], f32, tag="v1")
            # v1 = sqb/128 + EPS  (fold the LN epsilon in here)
            nc.vector.tensor_scalar(
                out=v1[:], in0=sqb[:], scalar1=1.0 / D, scalar2=EPS,
                op0=OP.mult, op1=OP.add,
            )
            var = apool.tile([128, F], f32, tag="var")
            nc.vector.scalar_tensor_tensor(
                out=var[:], in0=msq[:], scalar=-1.0 / (D * D), in1=v1[:],
                op0=OP.mult, op1=OP.add,
            )
            std = apool.tile([128, F], f32, tag="std")
            nc.scalar.activation(std[:], var[:], AF.Sqrt)
            rstd = apool.tile([128, F], f32, tag="rstd")
            nc.vector.reciprocal(rstd[:], std[:])
            core = apool.tile([128, F], f32, tag="core")
            # core = dot + sum * (-S/128)  (params[:,0] = -S/128)
            nc.vector.scalar_tensor_tensor(
                out=core[:], in0=sumb[:], scalar=params[:, 0:1], in1=dotb[:],
                op0=OP.mult, op1=OP.add,
            )
            v0 = apool.tile([128, F], f32, tag="v0")
            nc.vector.tensor_tensor(out=v0[:], in0=core[:], in1=rstd[:], op=OP.mult)
            vals = apool.tile([128, F], f32, tag="vals")
            # vals = v0 + b2p  (params[:,1] = beta@W2 + b2)
            nc.vector.tensor_scalar(
                out=vals[:], in0=v0[:], scalar1=params[:, 1:2], scalar2=None,
                op0=OP.add,
            )
            nc.sync.dma_start(vals_out[:], vals[:])

    _patch_serialization(nc)
    return nc


# ---------------------------------------------------------------------------
def _ensure_ntff_hook():
    """bass_utils' trace path imports antenv.axon_hooks, which this image's
    antenv package lacks. Synthesize it, backed by the boot module's ctypes
    NTFF profiler, so trace=True yields exec_time_ns."""
    import sys
    import types

    try:
        from antenv.axon_hooks import get_axon_ntff_profile_hook  # noqa: F401
        return
    except ImportError:
        pass
    hook = None
    try:
        from trn_agent_boot.trn_boot import _ntff_profile_via_ctypes

        hook = _ntff_profile_via_ctypes("/opt/axon/libaxon_pjrt.so")
    except Exception:
        hook = None
    mod = types.ModuleType("antenv.axon_hooks")
    mod._hook = hook
    mod.get_axon_ntff_profile_hook = lambda: mod._hook
    mod.set_axon_ntff_profile_hook = lambda h: setattr(mod, "_hook", h)
    sys.modules["antenv.axon_hooks"] = mod


def _mlp_device(dist, W1, b1, gamma, beta, W2, b2, trace=False):
    """Run the MLP part on the 8 NeuronCores. Returns vals [E] float32."""
    E = dist.shape[0]
    Ec = E // N_CORES                      # edges per core
    Eq = (Ec + RQ - 1) // RQ               # edges per quarter (unpadded)
    n_rounds = (Eq + NQ - 1) // NQ
    Epq = n_rounds * NQ                    # padded edges per quarter
    F = n_rounds * ROUND_E // 128

    W2v = W2[:, 0].astype(np.float64)
    W2p = (gamma.astype(np.float64) * W2v)
    S = float(W2p.sum())
    b2p = float(beta.astype(np.float64) @ W2v + b2[0])

    w1rep = np.tile(W1.astype(np.float32), (RQ, 1))            # [128, 128]
    cw = np.zeros((128, 2), np.float32)
    cw[:, 0] = 1.0
    cw[:, 1] = W2p.astype(np.float32)
    b1t = b1.astype(np.float32).reshape(128, 1)
    params = np.zeros((128, 2), np.float32)
    params[:, 0] = -S / D
    params[:, 1] = b2p

    in_maps = []
    for c in range(N_CORES):
        dc = dist[c * Ec:(c + 1) * Ec]                          # [Ec, 32]
        A = np.zeros((RQ, H, Epq), np.float32)
        for q in range(RQ):
            seg = dc[q * Eq:min((q + 1) * Eq, Ec)]
            A[q, :, :seg.shape[0]] = seg.T
        in_maps.append({
            "a": np.ascontiguousarray(A.reshape(128, Epq)),
            "w1rep": w1rep, "cw": cw, "b1t": b1t, "params": params,
        })

    if trace:
        _ensure_ntff_hook()
    nc = build_mlp_kernel(n_rounds)
    res = run_bass_kernel_spmd(
        nc, in_maps, core_ids=list(range(N_CORES)), trace=trace
    )

    vals = np.empty(E, np.float32)
    for c in range(N_CORES):
        v = res.results[c]["vals"]                              # [128, F]
        # [128, F] -> per-round blocks [128, 16] -> linear k=p*16+j within
        # round -> (quarter, n) -> quarter-major edge order
        fw = ROUND_E // 128
        Q = (v.reshape(128, n_rounds, fw)
              .transpose(1, 0, 2)            # [round, p, j]
              .reshape(n_rounds, RQ, NQ)     # [round, quarter, n]
              .transpose(1, 0, 2)            # [quarter, round, n]
              .reshape(RQ, Epq))
        vc = np.concatenate(
            [Q[q, :min((q + 1) * Eq, Ec) - q * Eq] for q in range(RQ)]
        )
        vals[c * Ec:(c + 1) * Ec] = vc
    return vals, res


def _coalesce(vals, edge_index):
    """to_undirected + coalesce(mean), matching the jax reference exactly.

    Arithmetic stays in edge_index's dtype: if the harness's jax has x64
    disabled, edge_index (and the reference's id math) is int32 and
    r*N_NODES+c wraps — mirror that exactly."""
    dt = edge_index.dtype
    row = edge_index[0]
    col = edge_index[1]
    E = row.shape[0]
    N = dt.type(N_NODES)
    with np.errstate(over="ignore"):
        ids = np.concatenate([row * N + col, col * N + row])
    v2 = np.concatenate([vals, vals]).astype(np.float64)
    uniq, inv = np.unique(ids, return_inverse=True)
    U = uniq.shape[0]
    sums = np.bincount(inv, weights=v2, minlength=U)
    counts = np.bincount(inv, minlength=U).astype(np.float64)
    means = (sums / np.maximum(counts, 1.0)).astype(np.float32)

    dist_out = np.zeros(2 * E, np.float32)
    dist_out[:U] = means
    uniq_pad = np.full(2 * E, -1, dt)
    uniq_pad[:U] = uniq
    ei_out = np.stack([uniq_pad // N, uniq_pad % N])
    return dist_out, ei_out


def kernel(dist, edge_index, W1, b1, gamma, beta, W2, b2, _trace=False):
    dist = np.asarray(dist, np.float32)
    edge_index = np.asarray(edge_index)
    W1 = np.asarray(W1, np.float32)
    b1 = np.asarray(b1, np.float32)
    gamma = np.asarray(gamma, np.float32)
    beta = np.asarray(beta, np.float32)
    W2 = np.asarray(W2, np.float32)
    b2 = np.asarray(b2, np.float32)

    vals, res = _mlp_device(dist, W1, b1, gamma, beta, W2, b2, trace=_trace)
    dist_out, ei_out = _coalesce(vals, edge_index)
    kernel.last_result = res
    return dist_out, ei_out


# revision 19
# speedup vs baseline: 2.1892x; 2.1892x over previous
"""Trainium2 Bass kernel for nn_DistanceHead (gnn_message_passing).

Pipeline per edge e:  vals[e] = LN(gelu(dist[e] @ W1 + b1)) @ (gamma*W2) + ...
followed by to_undirected coalesce (sort/unique over linearized edge ids).

Device strategy (8 NeuronCores, data-parallel over E):
  - Orientation "B": features (D=128) on partitions, edges on the free dim.
  - dist is pre-transposed on host into A_core[128, Epq]: row 32*q+k holds
    feature k of edge-quarter q. One DMA per round loads [128, 512] (2048
    edges = 4 quarters x 512).
  - mm1: out.T = W1.T @ dist.T per quarter via K=32 row-tiled matmuls
    (tile_position=(32q, 0)) with W1 replicated into all 4 row groups.
  - ACT applies erf-Gelu (+b1 per-partition bias) PSUM->SBUF.
  - DVE squares g (for the variance second moment).
  - LayerNorm + final dot are folded into three reductions over features:
    sum(g), dot(g, gamma*W2), sum(g^2), computed on the PE as matmuls with
    stationary [ones | gamma*W2] weights (M=2, col-tiled 4x).
  - Per-round stat rows are copied PSUM->SBUF (lane-preserving) and
    re-spread to a [128, F] per-edge layout with SBUF->SBUF DMAs.
  - Final assembly: vals = rstd*(dot - mu*S) + (beta@W2 + b2), computed once
    on [128, F] tiles.
Host does: input re-layout/sharding, gamma/beta folding, output un-permute,
and the global coalesce (sort/unique/segment-mean over 4M int64 ids).
"""

import numpy as np

import concourse.bass as bass
import concourse.mybir as mybir
from concourse.tile import TileContext
from concourse.vector_clock import ScopedClock
from concourse.bass_utils import run_bass_kernel_spmd

N_NODES = 100000
EPS = 1e-5
N_CORES = 8
H = 32          # input feature dim
D = 128         # hidden dim
NQ = 512        # edges per quarter per round
RQ = 4          # quarters (128 partitions / 32 features)
ROUND_E = NQ * RQ  # 2048 edges per round


# ---------------------------------------------------------------------------
# TileContext exit-drain fix: this toolchain's walrus rejects >1 sync wait on
# a Drain instruction ("Too many sync wait commands"). Split the exit drain's
# waits across one drain per semaphore.
def _drain_and_barrier_split(self, tick_clock, wait_clock):
    drain_inst = self.nc.sync.drain()
    wait_clock.add_sem_waits(
        drain_inst.ins, ScopedClock({None: tick_clock.global_clock})
    )
    si = drain_inst.ins.sync_info
    waits = list(si.on_wait) if si is not None else []
    if len(waits) > 1:
        drain_inst.ins.sync_info = mybir.SyncInfo(
            on_update=list(si.on_update), on_wait=[waits[0]]
        )
        for w in waits[1:]:
            extra = self.nc.sync.drain()
            extra.ins.sync_info = mybir.SyncInfo(on_update=[], on_wait=[w])
    self.nc.all_engine_barrier()
    popped = self.nc._tile_sem_poison_stack.pop()
    assert popped is self._sem_poison
    self.nc.clear_and_free_semaphores(list(self.sems.allocated().values()))
    self.nc.all_engine_barrier()


class _TileContextSplitDrain(TileContext):
    _drain_and_barrier = _drain_and_barrier_split


def _split_multi_waits(mod: dict) -> dict:
    """This walrus build rejects instructions carrying more than one sync
    wait ("Too many sync wait commands"). Hoist extra waits onto wait-only
    EventSemaphore instructions inserted just before, on the same engine
    queue (queues dispatch in order, so semantics are preserved)."""
    n = 0
    for f in mod["functions"]:
        for blk in f["blocks"]:
            new_insts = []
            for inst in blk["instructions"]:
                si = inst.get("sync_info")
                waits = (si or {}).get("on_wait") or []
                if len(waits) > 1:
                    for j, w in enumerate(waits[:-1]):
                        n += 1
                        new_insts.append({
                            "name": f"{inst['name']}_xw{j}",
                            "opcode": "EventSemaphore",
                            "engine": inst["engine"],
                            "ins": [],
                            "outs": [],
                            "debug": inst.get("debug"),
                            "sync_info": {"on_update": [], "on_wait": [w]},
                        })
                    si["on_wait"] = [waits[-1]]
                new_insts.append(inst)
            blk["instructions"] = new_insts
    return mod


def _patch_serialization(nc: bass.Bass) -> None:
    import orjson

    orig = nc.to_json_bytes

    def to_json_bytes_patched():
        return orjson.dumps(_split_multi_waits(orjson.loads(orig())))

    nc.to_json_bytes = to_json_bytes_patched


# ---------------------------------------------------------------------------
def build_mlp_kernel(n_rounds: int) -> bass.Bass:
    """Device kernel for one core: Epq = n_rounds*NQ edges per quarter.

    All PE streams are bf16 (fp32 rhs streams at half rate on TRN2, and
    bf16 weights enable FWL); PSUM accumulation stays fp32."""
    f32 = mybir.dt.float32
    bf16 = mybir.dt.bfloat16
    AF = mybir.ActivationFunctionType
    OP = mybir.AluOpType
    Epq = n_rounds * NQ
    F = n_rounds * ROUND_E // 128  # free size of per-edge stat buffers

    nc = bass.Bass()
    a_in = nc.dram_tensor("a", [128, Epq], bf16, kind="ExternalInput")
    w1rep_in = nc.dram_tensor("w1rep", [128, D], bf16, kind="ExternalInput")
    cw_in = nc.dram_tensor("cw", [128, 2], bf16, kind="ExternalInput")
    b1_in = nc.dram_tensor("b1t", [128, 1], f32, kind="ExternalInput")
    par_in = nc.dram_tensor("params", [128, 2], f32, kind="ExternalInput")
    vals_out = nc.dram_tensor("vals", [128, F], f32, kind="ExternalOutput")

    with _TileContextSplitDrain(nc) as tc:
        with (
            tc.tile_pool(name="consts", bufs=1) as cpool,
            tc.tile_pool(name="din", bufs=4) as dpool,
            tc.tile_pool(name="gbuf", bufs=2) as gpool,
            tc.tile_pool(name="g2buf", bufs=2) as g2pool,
            tc.tile_pool(name="stage", bufs=2) as spool,
            tc.tile_pool(name="stats", bufs=1) as stpool,
            tc.tile_pool(name="asm", bufs=1) as apool,
            tc.tile_pool(name="hp", bufs=3, space="PSUM") as hpool,
            tc.tile_pool(name="pstat", bufs=1, space="PSUM") as pspool,
        ):
            # Constants
            w1rep = cpool.tile([128, D], bf16)
            nc.sync.dma_start(w1rep[:], w1rep_in[:])
            cw = cpool.tile([128, 2], bf16)
            nc.sync.dma_start(cw[:], cw_in[:])
            b1t = cpool.tile([128, 1], f32)
            nc.sync.dma_start(b1t[:], b1_in[:])
            params = cpool.tile([128, 2], f32)
            nc.sync.dma_start(params[:], par_in[:])

            # Per-edge stat accumulation buffers [128, F]
            sumb = stpool.tile([128, F], f32, tag="sumb")
            dotb = stpool.tile([128, F], f32, tag="dotb")
            sqb = stpool.tile([128, F], f32, tag="sqb")

            for r in range(n_rounds):
                # ---- load dist tile [128, 512]: 4 quarters x 512 edges
                dt = dpool.tile([128, NQ], bf16, tag="din")
                nc.sync.dma_start(dt[:], a_in[:, r * NQ:(r + 1) * NQ])

                # ---- mm1: h.T per quarter (row-tiled K=32 matmuls)
                hpA = hpool.tile([128, 2 * NQ], f32, tag="hp")  # quarters 0,1
                hpB = hpool.tile([128, 2 * NQ], f32, tag="hp")  # quarters 2,3
                for q in range(RQ):
                    hp = hpA if q < 2 else hpB
                    col = (q % 2) * NQ
                    nc.tensor.matmul(
                        hp[:, col:col + NQ],
                        lhsT=w1rep[32 * q:32 * (q + 1), :],
                        rhs=dt[32 * q:32 * (q + 1), :],
                        tile_position=(32 * q, 0),
                    )

                # ---- gelu (+b1) PSUM -> SBUF, bf16 out
                gt = gpool.tile([128, ROUND_E], bf16, tag="gt")
                nc.scalar.activation(
                    gt[:, 0:2 * NQ], hpA[:], AF.Gelu, bias=b1t[:, 0:1]
                )
                nc.scalar.activation(
                    gt[:, 2 * NQ:4 * NQ], hpB[:], AF.Gelu, bias=b1t[:, 0:1]
                )

                # ---- square for variance (bf16 2x DVE mode)
                g2t = g2pool.tile([128, ROUND_E], bf16, tag="g2t")
                nc.vector.tensor_tensor(
                    out=g2t[:], in0=gt[:], in1=gt[:], op=OP.mult
                )

                # ---- stat matmuls: [ones | gamma*W2].T @ g -> [2, 512] per
                # quarter, col-tiled into one PSUM bank.
                gstat = pspool.tile([128, NQ], f32, tag="gstat")
                g2stat = pspool.tile([128, NQ], f32, tag="g2stat")
                # Interleave g/g2 stat matmuls so adjacent instructions hit
                # different array col-groups AND different PSUM banks,
                # letting the PE overlap them.
                for c in range(RQ):
                    c2 = (c + 1) % RQ
                    nc.tensor.matmul(
                        gstat[32 * c:32 * c + 2, :],
                        lhsT=cw[:, 0:2],
                        rhs=gt[:, NQ * c:NQ * (c + 1)],
                        tile_position=(0, 32 * c),
                    )
                    nc.tensor.matmul(
                        g2stat[32 * c2:32 * c2 + 1, :],
                        lhsT=cw[:, 0:1],
                        rhs=g2t[:, NQ * c2:NQ * (c2 + 1)],
                        tile_position=(0, 32 * c2),
                    )

                # ---- copy stat rows PSUM -> SBUF (lane-preserving)
                sg = spool.tile([128, NQ], f32, tag="sg")
                s2g = spool.tile([128, NQ], f32, tag="s2g")
                # sum rows {0,32,64,96}, dot rows {1,33,65,97}. Engines
                # need partition step 1, so copy the contiguous row range
                # (cost is free-dim driven; extra lanes are free).
                nc.scalar.copy(sg[0:98, :], gstat[0:98, :])
                nc.vector.tensor_copy(s2g[0:97, :], g2stat[0:97, :])

                # ---- re-spread to per-edge layout [128, 16] via SBUF DMA
                fo = r * (ROUND_E // 128)
                fw = ROUND_E // 128
                nc.gpsimd.dma_start(sumb[:, fo:fo + fw], sg[0:128:32, :])
                nc.gpsimd.dma_start(dotb[:, fo:fo + fw], sg[1:128:32, :])
                nc.gpsimd.dma_start(sqb[:, fo:fo + fw], s2g[0:128:32, :])

            # ---- final assembly on [128, F]
            msq = apool.tile([128, F], f32, tag="msq")
            nc.vector.tensor_tensor(out=msq[:], in0=sumb[:], in1=sumb[:], op=OP.mult)
            v1 = apool.tile([128, F], f32, tag="v1")
            # v1 = sqb/128 + EPS  (fold the LN epsilon in here)
            nc.vector.tensor_scalar(
                out=v1[:], in0=sqb[:], scalar1=1.0 / D, scalar2=EPS,
                op0=OP.mult, op1=OP.add,
            )
            var = apool.tile([128, F], f32, tag="var")
            nc.vector.scalar_tensor_tensor(
                out=var[:], in0=msq[:], scalar=-1.0 / (D * D), in1=v1[:],
                op0=OP.mult, op1=OP.add,
            )
            std = apool.tile([128, F], f32, tag="std")
            nc.scalar.activation(std[:], var[:], AF.Sqrt)
            rstd = apool.tile([128, F], f32, tag="rstd")
            nc.vector.reciprocal(rstd[:], std[:])
            core = apool.tile([128, F], f32, tag="core")
            # core = dot + sum * (-S/128)  (params[:,0] = -S/128)
            nc.vector.scalar_tensor_tensor(
                out=core[:], in0=sumb[:], scalar=params[:, 0:1], in1=dotb[:],
                op0=OP.mult, op1=OP.add,
            )
            v0 = apool.tile([128, F], f32, tag="v0")
            nc.vector.tensor_tensor(out=v0[:], in0=core[:], in1=rstd[:], op=OP.mult)
            vals = apool.tile([128, F], f32, tag="vals")
            # vals = v0 + b2p  (params[:,1] = beta@W2 + b2)
            nc.vector.tensor_scalar(
                out=vals[:], in0=v0[:], scalar1=params[:, 1:2], scalar2=None,
                op0=OP.add,
            )
            nc.sync.dma_start(vals_out[:], vals[:])

    _patch_serialization(nc)
    return nc


# ---------------------------------------------------------------------------
def _ensure_ntff_hook():
    """bass_utils' trace path imports antenv.axon_hooks, which this image's
    antenv package lacks. Synthesize it, backed by the boot module's ctypes
    NTFF profiler, so trace=True yields exec_time_ns."""
    import sys
    import types

    try:
        from antenv.axon_hooks import get_axon_ntff_profile_hook  # noqa: F401
        return
    except ImportError:
        pass
    hook = None
    try:
        from trn_agent_boot.trn_boot import _ntff_profile_via_ctypes

        hook = _ntff_profile_via_ctypes("/opt/axon/libaxon_pjrt.so")
    except Exception:
        hook = None
    mod = types.ModuleType("antenv.axon_hooks")
    mod._hook = hook
    mod.get_axon_ntff_profile_hook = lambda: mod._hook
    mod.set_axon_ntff_profile_hook = lambda h: setattr(mod, "_hook", h)
    sys.modules["antenv.axon_hooks"] = mod


def _mlp_device(dist, W1, b1, gamma, beta, W2, b2, trace=False):
    """Run the MLP part on the 8 NeuronCores. Returns vals [E] float32."""
    E = dist.shape[0]
    Ec = E // N_CORES                      # edges per core
    Eq = (Ec + RQ - 1) // RQ               # edges per quarter (unpadded)
    n_rounds = (Eq + NQ - 1) // NQ
    Epq = n_rounds * NQ                    # padded edges per quarter
    F = n_rounds * ROUND_E // 128

    import ml_dtypes

    bf16 = ml_dtypes.bfloat16
    W2v = W2[:, 0].astype(np.float64)
    W2p = (gamma.astype(np.float64) * W2v)
    # The device dot uses the bf16-rounded weights; S must match them.
    W2p_bf = W2p.astype(np.float32).astype(bf16)
    S = float(W2p_bf.astype(np.float64).sum())
    b2p = float(beta.astype(np.float64) @ W2v + b2[0])

    w1rep = np.tile(W1.astype(np.float32), (RQ, 1)).astype(bf16)  # [128, 128]
    cw = np.zeros((128, 2), bf16)
    cw[:, 0] = 1.0
    cw[:, 1] = W2p_bf
    b1t = b1.astype(np.float32).reshape(128, 1)
    params = np.zeros((128, 2), np.float32)
    params[:, 0] = -S / D
    params[:, 1] = b2p

    in_maps = []
    for c in range(N_CORES):
        dc = dist[c * Ec:(c + 1) * Ec]                          # [Ec, 32]
        A = np.zeros((RQ, H, Epq), bf16)
        for q in range(RQ):
            seg = dc[q * Eq:min((q + 1) * Eq, Ec)]
            A[q, :, :seg.shape[0]] = seg.T.astype(bf16)
        in_maps.append({
            "a": np.ascontiguousarray(A.reshape(128, Epq)),
            "w1rep": w1rep, "cw": cw, "b1t": b1t, "params": params,
        })

    if trace:
        _ensure_ntff_hook()
    nc = build_mlp_kernel(n_rounds)
    res = run_bass_kernel_spmd(
        nc, in_maps, core_ids=list(range(N_CORES)), trace=trace
    )

    vals = np.empty(E, np.float32)
    for c in range(N_CORES):
        v = res.results[c]["vals"]                              # [128, F]
        # [128, F] -> per-round blocks [128, 16] -> linear k=p*16+j within
        # round -> (quarter, n) -> quarter-major edge order
        fw = ROUND_E // 128
        Q = (v.reshape(128, n_rounds, fw)
              .transpose(1, 0, 2)            # [round, p, j]
              .reshape(n_rounds, RQ, NQ)     # [round, quarter, n]
              .transpose(1, 0, 2)            # [quarter, round, n]
              .reshape(RQ, Epq))
        vc = np.concatenate(
            [Q[q, :min((q + 1) * Eq, Ec) - q * Eq] for q in range(RQ)]
        )
        vals[c * Ec:(c + 1) * Ec] = vc
    return vals, res


def _coalesce(vals, edge_index):
    """to_undirected + coalesce(mean), matching the jax reference exactly.

    Arithmetic stays in edge_index's dtype: if the harness's jax has x64
    disabled, edge_index (and the reference's id math) is int32 and
    r*N_NODES+c wraps — mirror that exactly."""
    dt = edge_index.dtype
    row = edge_index[0]
    col = edge_index[1]
    E = row.shape[0]
    N = dt.type(N_NODES)
    with np.errstate(over="ignore"):
        ids = np.concatenate([row * N + col, col * N + row])
    v2 = np.concatenate([vals, vals]).astype(np.float64)
    uniq, inv = np.unique(ids, return_inverse=True)
    U = uniq.shape[0]
    sums = np.bincount(inv, weights=v2, minlength=U)
    counts = np.bincount(inv, minlength=U).astype(np.float64)
    means = (sums / np.maximum(counts, 1.0)).astype(np.float32)

    dist_out = np.zeros(2 * E, np.float32)
    dist_out[:U] = means
    uniq_pad = np.full(2 * E, -1, dt)
    uniq_pad[:U] = uniq
    # Decode ids -> (row, col). XLA's int32 floor-divide on negative
    # (wrapped) ids differs from numpy's; run the decode through jax on CPU
    # so it bit-matches the reference.
    ei_out = None
    if np.issubdtype(dt, np.signedinteger) and uniq_pad.min() < 0:
        try:
            import jax
            import jax.numpy as jnp

            with jax.default_device(jax.devices("cpu")[0]):
                u = jnp.asarray(uniq_pad)
                ei_out = np.stack(
                    [np.asarray(u // N_NODES), np.asarray(u % N_NODES)]
                ).astype(dt)
        except Exception:
            ei_out = None
    if ei_out is None:
        ei_out = np.stack([uniq_pad // N, uniq_pad % N])
    return dist_out, ei_out


def kernel(dist, edge_index, W1, b1, gamma, beta, W2, b2, _trace=False):
    dist = np.asarray(dist, np.float32)
    edge_index = np.asarray(edge_index)
    W1 = np.asarray(W1, np.float32)
    b1 = np.asarray(b1, np.float32)
    gamma = np.asarray(gamma, np.float32)
    beta = np.asarray(beta, np.float32)
    W2 = np.asarray(W2, np.float32)
    b2 = np.asarray(b2, np.float32)

    vals, res = _mlp_device(dist, W1, b1, gamma, beta, W2, b2, trace=_trace)
    dist_out, ei_out = _coalesce(vals, edge_index)
    kernel.last_result = res
    return dist_out, ei_out
